# revision 20
# baseline (speedup 1.0000x reference)
"""Trainium2 Bass kernel for nn_BiLinearMHSLayer.

Reference computation (per batch element b):
    t  = x @ fc_w.T + fc_b            [S, E]      (S=1024, IN=768, E=256)
    bl = (t @ bi_w.T).reshape(S,L,E) + bias       (L=12)
    out[i,l,j] = sum_e bl[i,l,e] * t[j,e]         [S, L, S]

Sharding: data-parallel over batch B=8 -> one batch element per NeuronCore.

Per-core dataflow (everything kept in "transposed" layout so the contraction
dim lands on SBUF partitions for the PE-array matmuls; x / fc_w / bi_w are
pre-transposed AND pre-cast to bf16 on the HOST so the device only does
straight same-dtype strided loads and the PE runs pure matmuls):
    xT   [IN, S] DMA load of host-transposed bf16 x
    tT   [E, S]  = fc_wT.T @ xT  + fc_b          (24 matmuls,  N=512)
    blT  [E*L,S] = bi_wT.T @ tT  + bias          (96 matmuls,  N=512)
    out  (per l) = blT_l.T @ tT                  (384 matmuls, N=512)

All PSUM tiles are full [128,1024] 2-bank tiles (a matmul's output is capped
at one bank = 512 fp32, so each tile takes matmul pairs) evacuated by single
full-width instructions.  blT f-tiles 0-11 are produced first so l-half-0
output units for all 8 i-tiles ship while f-tiles 12-23 are still being
computed.  PSUM->SBUF evacuation alternates between the Vector and Scalar
engines.

The output is stored bf16 (halving the dominant HBM write stream to 24MB/core)
and upcast to fp32 on the host.  Operands are bf16 (fp32 accumulation in
PSUM); |err| vs the fp32 reference is ~5.2e-3 of max|out| (vs the 2e-2
gate).  HW A/B showed the output DMA stream is fully hidden; weight/x
transposes stay on the host upload path.
"""

import json

import numpy as np

import concourse.bass as bass
import concourse.mybir as mybir
import concourse.tile as tile
from concourse.bass_utils import run_bass_kernel_spmd

B, S, IN, E, L = 8, 1024, 768, 256, 12
N_CORES = 8
FP32 = mybir.dt.float32
BF16 = mybir.dt.bfloat16
ACT_COPY = mybir.ActivationFunctionType.Copy
ACT_IDENT = mybir.ActivationFunctionType.Identity

# ---------------------------------------------------------------------------
# Workaround: walrus on this image rejects instructions carrying more than one
# embedded sem wait ("Too many sync wait commands", CoreV3GenImpl
# setupSyncWait).  Split excess waits onto EventSemaphore instructions
# inserted immediately before, on the same engine (identical semantics: the
# waits execute, in order, before the instruction).
_WAIT_CAPS = {}
_DEFAULT_WAIT_CAP = 1


def _dedup_ldweights(j) -> int:
    """Drop an Ldweights whose weights operand is identical to the previous
    PE Ldweights when exactly one Matmult sits between them on the PE queue
    (the lowering emits one Ldweights per matmul even for back-to-back
    matmuls sharing the stationary operand).  The dropped instruction's
    waits must be implied by waits the PE queue already executed within the
    pair (same sem, <= value) -- sem values are monotone, so re-waiting is
    vacuous.  The surviving Matmult is non-self-loading and keeps using the
    already-loaded weights: identical semantics, one less PE instruction."""
    ndrop = 0
    for f in j.get("functions", []):
        for bb in f.get("blocks", []):
            insts = bb.get("instructions", [])
            # state: last kept PE Ldweights (key, waits seen in its group)
            prev_key = None
            prev_waits = {}
            n_mm_since = 0
            keep = []
            for inst in insts:
                if inst.get("engine") != "PE":
                    keep.append(inst)
                    continue
                op = inst.get("opcode")
                waits = ((inst.get("sync_info") or {}).get("on_wait")) or []
                if op == "Ldweights":
                    key = json.dumps(inst.get("ins"), sort_keys=True)
                    implied = (
                        prev_key == key
                        and n_mm_since == 1
                        and not ((inst.get("sync_info") or {}).get("on_update"))
                        and all(
                            w.get("sync_type") == "semaphore"
                            and w.get("wait_mode") == "sem-ge-imm"
                            and w.get("id") in prev_waits
                            and w.get("wait_value", 0) <= prev_waits[w["id"]]
                            for w in waits
                        )
                    )
                    if implied:
                        ndrop += 1
                        n_mm_since = 0
                        continue
                    prev_key = key
                    prev_waits = {}
                    n_mm_since = 0
                    for w in waits:
                        if w.get("sync_type") == "semaphore" and w.get(
                                "wait_mode") == "sem-ge-imm":
                            prev_waits[w["id"]] = max(
                                prev_waits.get(w["id"], 0), w.get("wait_value", 0))
                    keep.append(inst)
                elif op == "Matmult":
                    n_mm_since += 1
                    for w in waits:
                        if w.get("sync_type") == "semaphore" and w.get(
                                "wait_mode") == "sem-ge-imm":
                            prev_waits[w["id"]] = max(
                                prev_waits.get(w["id"], 0), w.get("wait_value", 0))
                    keep.append(inst)
                else:
                    # any other PE instruction breaks the pairing window
                    prev_key = None
                    prev_waits = {}
                    n_mm_since = 0
                    keep.append(inst)
            bb["instructions"] = keep
    return ndrop


def _drop_vacuous_waits(j) -> int:
    """Remove sem waits already implied by an earlier wait on the same
    engine queue (same sem, >= value).  Sem values are monotone
    non-decreasing and never reset mid-program, and each engine executes
    its queue in order, so re-waiting on an already-passed threshold is a
    no-op.  EventSemaphore instructions left with no waits are dropped."""
    ndrop = 0
    for f in j.get("functions", []):
        for bb in f.get("blocks", []):
            seen = {}  # (engine, sem id) -> max wait_value already executed
            keep = []
            for inst in bb.get("instructions", []):
                eng = inst.get("engine")
                si = inst.get("sync_info") or {}
                waits = si.get("on_wait") or []
                kept_waits = []
                for w in waits:
                    if (w.get("sync_type") == "semaphore"
                            and w.get("wait_mode") == "sem-ge-imm"):
                        k = (eng, w["id"])
                        v = w.get("wait_value", 0)
                        if v <= seen.get(k, -1):
                            ndrop += 1
                            continue
                        seen[k] = v
                    kept_waits.append(w)
                if si:
                    si["on_wait"] = kept_waits
                if (inst.get("opcode") == "EventSemaphore" and not kept_waits
                        and not (si.get("on_update") or [])):
                    continue
                keep.append(inst)
            bb["instructions"] = keep
    return ndrop


# The remote executor faults at runtime when back-to-back same-weights
# matmuls share one Ldweights (NEFF compiles, execution dies), so the
# dedup/vacuous passes stay disabled.
_LDW_DEDUP = False


def _fix_sync_waits(blob: bytes) -> bytes:
    j = json.loads(blob)
    if _LDW_DEDUP:
        _drop_vacuous_waits(j)
        _dedup_ldweights(j)
    n = 0
    for f in j.get("functions", []):
        for bb in f.get("blocks", []):
            out = []
            for inst in bb.get("instructions", []):
                si = inst.get("sync_info")
                waits = (si or {}).get("on_wait") or []
                cap = _WAIT_CAPS.get(inst.get("opcode"), _DEFAULT_WAIT_CAP)
                if len(waits) > cap:
                    excess, keep = waits[:-cap], waits[-cap:]
                    for w in excess:
                        n += 1
                        out.append({
                            "debug": inst.get("debug", 0),
                            "engine": inst["engine"],
                            "ins": [],
                            "name": f"waitsplit-{n}",
                            "opcode": "EventSemaphore",
                            "outs": [],
                            "sync_info": {"on_update": [], "on_wait": [w]},
                        })
                    si["on_wait"] = keep
                out.append(inst)
            bb["instructions"] = out
    return json.dumps(j).encode()


# ---------------------------------------------------------------------------
_DMA_SRC_CONST = False  # debug ablation: output DMAs read a constant tile
_SKIP_OUT_DMA = False   # debug ablation: no output DMAs (PE/evac floor)
_EVAC_PAT = "VA"        # evacuation engine rotation: V=DVE, A=ACT
                        # (Pool/gpsimd cannot access PSUM -- walrus birverifier)
_DMA_RINGS = 2          # rotate output stores across SP HWDGE / Pool SWDGE
_STG_BUFS = 3           # output staging buffers (store pipeline depth)
_PAIR_L = False         # score waves: two l's per 4-bank PSUM tile, one
                        # [128,2048] evacuation per pair (halves score evac
                        # instruction count, halves PSUM lookahead to 2)
_SKIP_IN_DMA = False    # debug ablation: no x/fc_w/bi_w loads (timing only)
_TBL_ACT = False        # tT/blT evacs pinned to ACT: wave matmuls then wait
                        # on ONE monotone ACT sem for blT/tT readiness
                        # instead of alternating DVE/ACT sems (fewer split
                        # EventSemaphore instructions on the PE queue)


def _emit_consts(nc, const_pool):
    stg_const = None
    if _DMA_SRC_CONST:
        stg_const = const_pool.tile([128, 6 * 512], BF16, tag="stg_const")
        nc.vector.memset(stg_const[:], 1.0)
    return stg_const


def _emit_body(nc, tc, pools, dram, ctr, consts):
    """Emit one full per-core computation."""
    x_d, fcw_d, fcb_d, biw_d, bias_d, out_d = dram
    (const_pool, big_pool, in_pool, psum_s, psum_w, stg_pool, dram_pool) = pools
    stg_const = consts

    def evac(dst_ap, src_ap, bias_ap=None, force_act=False):
        """PSUM -> SBUF copy (+ optional per-partition bias add), rotated
        across DVE / ACT / Pool per _EVAC_PAT (weighted by engine rates)."""
        c = ctr[0]
        ctr[0] += 1
        eng = "A" if force_act else _EVAC_PAT[c % len(_EVAC_PAT)]
        if eng == "A":
            if bias_ap is not None:
                # Copy doesn't accept an AP bias; Identity does.
                nc.scalar.activation(dst_ap, src_ap, ACT_IDENT, bias=bias_ap)
            else:
                nc.scalar.activation(dst_ap, src_ap, ACT_COPY)
        else:
            if bias_ap is not None:
                nc.vector.tensor_scalar_add(dst_ap, src_ap, bias_ap)
            else:
                nc.vector.tensor_copy(dst_ap, src_ap)

    # ---- persistent SBUF tensors -------------------------------------------
    fcb_sb = const_pool.tile([128, 2], FP32, tag="fcb_sb")      # col ec: fc_b[ec*128+p]
    bias_sb = const_pool.tile([128, 2], FP32, tag="bias_sb")
    xT = big_pool.tile([128, 6 * 1024], BF16, tag="xT")         # [i%128, (i/128, s)]
    fcwT = big_pool.tile([128, 6 * 256], BF16, tag="fcwT")      # [i%128, (i/128, e)]
    biwT = big_pool.tile([128, 2 * 3072], BF16, tag="biwT")     # [e%128, (e/128, f)]
    tT = big_pool.tile([128, 2 * 1024], BF16, tag="tT")         # [e%128, (e/128, s)]
    blT = big_pool.tile([128, 24 * 1024], BF16, tag="blT")      # [f%128, (f/128, s)]

    # ---- input loads --------------------------------------------------------
    # x / fc_w / bi_w arrive HOST-pre-transposed AND pre-cast to bf16 in
    # DRAM (xT [IN,S], fc_wT [IN,E], bi_wT [E,E*L] -- kernel() marshals
    # them), so the transposed SBUF layouts are produced by straight
    # same-dtype DMA loads: no PE transposes, no PSUM round trip, no
    # evacuation copies, half the load bytes of the fp32 upload, and no
    # gpsimd-only cast restriction -- the loads split across the SP HWDGE
    # and Pool SWDGE queues.  Order = startup critical path: x (gates tT),
    # fc_w, bi_w f-half 0 (gates blT f 0-11).
    xT_dst = xT[:].rearrange("p (ic s) -> p ic s", ic=6)
    x_src = x_d.rearrange("(ic p) s -> p ic s", p=128)
    biwT_dst = biwT[:].rearrange("p (kc f) -> p kc f", kc=2)
    biw_src = biw_d.rearrange("(kc p) f -> p kc f", p=128)
    if not _SKIP_IN_DMA:
        nc.sync.dma_start(out=xT_dst[:, :, 0:512], in_=x_src[:, :, 0:512])
        nc.gpsimd.dma_start(
            out=fcwT[:].rearrange("p (ic e) -> p ic e", ic=6),
            in_=fcw_d.rearrange("(ic p) e -> p ic e", p=128))
        nc.sync.dma_start(out=xT_dst[:, :, 512:1024], in_=x_src[:, :, 512:1024])
        nc.gpsimd.dma_start(out=biwT_dst[:, :, 0:1536], in_=biw_src[:, :, 0:1536])
        nc.gpsimd.dma_start(out=biwT_dst[:, :, 1536:3072], in_=biw_src[:, :, 1536:3072])
    nc.sync.dma_start(
        out=fcb_sb[:], in_=fcb_d.rearrange("(c p) one -> p (c one)", p=128))
    nc.sync.dma_start(
        out=bias_sb[:], in_=bias_d.rearrange("(c p) one -> p (c one)", p=128))

    # ---- building blocks ----------------------------------------------------
    # Matmul moving size is N=512: the ISA caps a matmul's PSUM output at one
    # bank (512 fp32).  PSUM tiles are full [128,1024] 2-bank tiles written
    # by matmul pairs (one per bank) so every evacuation is a single
    # full-width [128,1024] instruction.
    def biwT_col(ft, kc):
        return kc * 3072 + ft * 128

    def emit_tT(ec):
        p = psum_s.tile([128, 1024], FP32, tag="pmm")
        for ic in range(6):
            for sh in range(2):
                nc.tensor.matmul(
                    p[:, sh * 512:(sh + 1) * 512],
                    fcwT[:, ic * 256 + ec * 128:ic * 256 + (ec + 1) * 128],
                    xT[:, ic * 1024 + sh * 512:ic * 1024 + (sh + 1) * 512],
                    start=(ic == 0), stop=(ic == 5))
        evac(tT[:, ec * 1024:(ec + 1) * 1024], p[:],
             bias_ap=fcb_sb[:, ec:ec + 1], force_act=_TBL_ACT)

    def emit_blT(fts):
        for ft in fts:
            p = psum_s.tile([128, 1024], FP32, tag="pmm")
            for kc in range(2):
                for sh in range(2):
                    nc.tensor.matmul(
                        p[:, sh * 512:(sh + 1) * 512],
                        biwT[:, biwT_col(ft, kc):biwT_col(ft, kc) + 128],
                        tT[:, kc * 1024 + sh * 512:kc * 1024 + (sh + 1) * 512],
                        start=(kc == 0), stop=(kc == 1))
            evac(blT[:, ft * 1024:(ft + 1) * 1024], p[:],
                 bias_ap=bias_sb[:, ft % 2:ft % 2 + 1], force_act=_TBL_ACT)

    def out_dma(out_ap, in_ap):
        # Rotate output stores across independent descriptor-generation
        # paths (SP HWDGE and the otherwise-idle Pool SWDGE) so trigger /
        # completion handling of consecutive stores proceeds in parallel.
        # ACT is deliberately excluded: a dma trigger's sem-wait executes
        # in-order on the issuing queue and would stall ACT's evac copies.
        if _SKIP_OUT_DMA:
            return
        engines = [nc.sync, nc.gpsimd][:max(1, _DMA_RINGS)]
        eng = engines[ctr[1] % len(engines)]
        ctr[1] += 1
        eng.dma_start(out=out_ap, in_=in_ap)

    def emit_wave(its, lhs=(0, 1), tail_split=False):
        # output unit = (i-tile, l-half) x FULL j: [128 i, 6 l, 1024 j].
        # Full-j units make every partition's DRAM write one contiguous 12KB
        # run -- HW probe showed 2KB-granular strided writes sustain only
        # ~half the bandwidth of contiguous runs.  One l per 2-bank PSUM
        # tile written by four N=512 matmuls (kc accumulation x j-halves),
        # single [128,1024] evacuation.
        # tail_split: ship the last unit as two 3-label DMAs so the final
        # drain overlaps the last evacuations.
        for it in its:
            for lh in lhs:
                last = tail_split and it == its[-1] and lh == lhs[-1]
                stg = stg_pool.tile([128, 6 * 1024], BF16, tag="stg")
                npair = 2 if _PAIR_L else 1
                for lp in range(0, 6, npair):
                    p = psum_w.tile([128, npair * 1024], FP32, tag="pmm")
                    for li in range(npair):
                        l = lh * 6 + lp + li
                        # kc outer: each blT weight tile is loaded once and
                        # streams both j-halves (half the LDWEIGHTS traffic)
                        for kc in range(2):
                            ft = 2 * l + kc
                            for jh in range(2):
                                c0 = li * 1024 + jh * 512
                                nc.tensor.matmul(
                                    p[:, c0:c0 + 512],
                                    blT[:, ft * 1024 + it * 128:ft * 1024 + (it + 1) * 128],
                                    tT[:, kc * 1024 + jh * 512:kc * 1024 + (jh + 1) * 512],
                                    start=(kc == 0), stop=(kc == 1))
                    evac(stg[:, lp * 1024:(lp + npair) * 1024], p[:])
                    cut = 4 if _PAIR_L else 3
                    if last and lp + npair == cut:
                        out_dma(
                            out_d[it * 128:(it + 1) * 128, lh * 6:lh * 6 + cut, :],
                            stg[:, 0:cut * 1024].rearrange("p (l j) -> p l j", l=cut))
                if last:
                    cut = 4 if _PAIR_L else 3
                    out_dma(
                        out_d[it * 128:(it + 1) * 128, lh * 6 + cut:lh * 6 + 6, :],
                        stg[:, cut * 1024:].rearrange("p (l j) -> p l j", l=6 - cut))
                else:
                    out_dma(
                        out_d[it * 128:(it + 1) * 128, lh * 6:lh * 6 + 6, :],
                        stg[:].rearrange("p (l j) -> p l j", l=6))

    # ---- schedule -----------------------------------------------------------
    # tT is produced full-width (both e-halves), then blT f-tiles 0-11
    # (l-half 0 weights) so l-half-0 waves for all 8 i-tiles ship while
    # f-tiles 12-23 are still being produced.
    emit_tT(0)
    emit_tT(1)
    emit_blT(range(0, 12))
    emit_wave((0, 1, 2, 3), lhs=(0,))
    emit_blT(range(12, 24))
    emit_wave((4, 5, 6, 7), lhs=(0,))
    emit_wave((0, 1, 2, 3), lhs=(1,))
    emit_wave((4, 5, 6, 7), lhs=(1,), tail_split=True)


def build_nc(unroll: int = 1):
    """Build the Bass program.  unroll>1 repeats the whole body (for timing
    measurements via wall-clock differencing)."""
    nc = bass.Bass(trn_type="TRN2")
    # x / fc_w / bi_w are uploaded host-pre-transposed and pre-cast to bf16
    # (see _emit_body loads)
    x_d = nc.dram_tensor("x", [IN, S], BF16, kind="ExternalInput")
    fcw_d = nc.dram_tensor("fc_w", [IN, E], BF16, kind="ExternalInput")
    fcb_d = nc.dram_tensor("fc_b", [E, 1], FP32, kind="ExternalInput")
    biw_d = nc.dram_tensor("bi_w", [E, E * L], BF16, kind="ExternalInput")
    bias_d = nc.dram_tensor("bias", [E, 1], FP32, kind="ExternalInput")
    # Output is stored bf16 (halves the dominant HBM write stream); the host
    # upcasts to fp32.  Quantization adds ~1e-3 rel err on top of the ~4e-3
    # bf16-compute error -- well inside the 2e-2 gate.
    out_d = nc.dram_tensor("out", [S, L, S], BF16, kind="ExternalOutput")
    dram = (x_d, fcw_d, fcb_d, biw_d, bias_d, out_d)

    with tile.TileContext(nc) as tc:
        with (
            tc.tile_pool(name="const", bufs=1) as const_pool,
            tc.tile_pool(name="big", bufs=1) as big_pool,
            tc.tile_pool(name="inp", bufs=1) as in_pool,
            tc.tile_pool(name="psum_mm", bufs=2 if _PAIR_L else 4,
                         space="PSUM") as psum_mm,
            tc.tile_pool(name="stg", bufs=_STG_BUFS) as stg_pool,
            tc.tile_pool(name="dram", bufs=1, space="DRAM") as dram_pool,
        ):
            pools = (const_pool, big_pool, in_pool, psum_mm, psum_mm, stg_pool,
                     dram_pool)
            ctr = [0, 0]
            consts = _emit_consts(nc, const_pool)
            for _ in range(unroll):
                _emit_body(nc, tc, pools, dram, ctr, consts)

    blob = _fix_sync_waits(nc.to_json_bytes())
    nc.to_json_bytes = lambda: blob
    return nc


_CACHE = {}


def _get_nc(unroll: int = 1):
    if unroll not in _CACHE:
        _CACHE[unroll] = build_nc(unroll)
    return _CACHE[unroll]


def kernel(input_tensor, fc_w, fc_b, bi_w, bias):
    # Host-side input marshaling: per-core batch slice of x, and x/fc_w/bi_w
    # pre-transposed AND pre-cast to bf16 so the device consumes them with
    # straight same-dtype strided loads (host cast is bitwise-identical to
    # the previous on-device DMA cast).
    import ml_dtypes
    bf16 = ml_dtypes.bfloat16
    input_tensor = np.asarray(input_tensor, dtype=np.float32)
    fcw_t = np.ascontiguousarray(np.asarray(fc_w, dtype=np.float32).T.astype(bf16))
    fc_b = np.ascontiguousarray(np.asarray(fc_b, dtype=np.float32)).reshape(E, 1)
    biw_t = np.ascontiguousarray(np.asarray(bi_w, dtype=np.float32).T.astype(bf16))
    bias = np.ascontiguousarray(np.asarray(bias, dtype=np.float32)).reshape(E, 1)
    assert input_tensor.shape == (B, S, IN)

    nc = _get_nc()
    in_maps = [
        {"x": np.ascontiguousarray(input_tensor[c].T.astype(bf16)), "fc_w": fcw_t,
         "fc_b": fc_b, "bi_w": biw_t, "bias": bias}
        for c in range(N_CORES)
    ]
    res = run_bass_kernel_spmd(nc, in_maps, core_ids=list(range(N_CORES)))
    return np.stack(
        [np.asarray(res.results[c]["out"]) for c in range(N_CORES)], axis=0
    ).astype(np.float32)



# revision 21
# speedup vs baseline: 1.1644x; 1.1644x over previous
"""Trainium2 Bass kernel for nn_BiLinearMHSLayer.

Reference computation (per batch element b):
    t  = x @ fc_w.T + fc_b            [S, E]      (S=1024, IN=768, E=256)
    bl = (t @ bi_w.T).reshape(S,L,E) + bias       (L=12)
    out[i,l,j] = sum_e bl[i,l,e] * t[j,e]         [S, L, S]

Sharding: data-parallel over batch B=8 -> one batch element per NeuronCore.

Per-core dataflow (everything kept in "transposed" layout so the contraction
dim lands on SBUF partitions for the PE-array matmuls; x / fc_w / bi_w are
pre-transposed AND pre-cast to bf16 on the HOST so the device only does
straight same-dtype strided loads and the PE runs pure matmuls):
    xT   [IN, S] DMA load of host-transposed bf16 x
    tT   [E, S]  = fc_wT.T @ xT  + fc_b          (24 matmuls,  N=512)
    blT  [E*L,S] = bi_wT.T @ tT  + bias          (96 matmuls,  N=512)
    out  (per l) = blT_l.T @ tT                  (384 matmuls, N=512)

All PSUM tiles are full [128,1024] 2-bank tiles (a matmul's output is capped
at one bank = 512 fp32, so each tile takes matmul pairs) evacuated by single
full-width instructions.  blT f-tiles 0-11 are produced first so l-half-0
output units for all 8 i-tiles ship while f-tiles 12-23 are still being
computed.  PSUM->SBUF evacuation alternates between the Vector and Scalar
engines.

The output is stored bf16 (halving the dominant HBM write stream to 24MB/core)
and upcast to fp32 on the host.  Operands are bf16 (fp32 accumulation in
PSUM); |err| vs the fp32 reference is ~5.2e-3 of max|out| (vs the 2e-2
gate).  HW A/B showed the output DMA stream is fully hidden; weight/x
transposes stay on the host upload path.
"""

import json

import numpy as np

import concourse.bass as bass
import concourse.mybir as mybir
import concourse.tile as tile
from concourse.bass_utils import run_bass_kernel_spmd

B, S, IN, E, L = 8, 1024, 768, 256, 12
N_CORES = 8
FP32 = mybir.dt.float32
BF16 = mybir.dt.bfloat16
ACT_COPY = mybir.ActivationFunctionType.Copy
ACT_IDENT = mybir.ActivationFunctionType.Identity

# ---------------------------------------------------------------------------
# Workaround: walrus on this image rejects instructions carrying more than one
# embedded sem wait ("Too many sync wait commands", CoreV3GenImpl
# setupSyncWait).  Split excess waits onto EventSemaphore instructions
# inserted immediately before, on the same engine (identical semantics: the
# waits execute, in order, before the instruction).
_WAIT_CAPS = {}
_DEFAULT_WAIT_CAP = 1


def _dedup_ldweights(j) -> int:
    """Drop an Ldweights whose weights operand is identical to the previous
    PE Ldweights when exactly one Matmult sits between them on the PE queue
    (the lowering emits one Ldweights per matmul even for back-to-back
    matmuls sharing the stationary operand).  The dropped instruction's
    waits must be implied by waits the PE queue already executed within the
    pair (same sem, <= value) -- sem values are monotone, so re-waiting is
    vacuous.  The surviving Matmult is non-self-loading and keeps using the
    already-loaded weights: identical semantics, one less PE instruction."""
    ndrop = 0
    for f in j.get("functions", []):
        for bb in f.get("blocks", []):
            insts = bb.get("instructions", [])
            # state: last kept PE Ldweights (key, waits seen in its group)
            prev_key = None
            prev_waits = {}
            n_mm_since = 0
            keep = []
            for inst in insts:
                if inst.get("engine") != "PE":
                    keep.append(inst)
                    continue
                op = inst.get("opcode")
                waits = ((inst.get("sync_info") or {}).get("on_wait")) or []
                if op == "Ldweights":
                    key = json.dumps(inst.get("ins"), sort_keys=True)
                    implied = (
                        prev_key == key
                        and n_mm_since == 1
                        and not ((inst.get("sync_info") or {}).get("on_update"))
                        and all(
                            w.get("sync_type") == "semaphore"
                            and w.get("wait_mode") == "sem-ge-imm"
                            and w.get("id") in prev_waits
                            and w.get("wait_value", 0) <= prev_waits[w["id"]]
                            for w in waits
                        )
                    )
                    if implied:
                        ndrop += 1
                        n_mm_since = 0
                        continue
                    prev_key = key
                    prev_waits = {}
                    n_mm_since = 0
                    for w in waits:
                        if w.get("sync_type") == "semaphore" and w.get(
                                "wait_mode") == "sem-ge-imm":
                            prev_waits[w["id"]] = max(
                                prev_waits.get(w["id"], 0), w.get("wait_value", 0))
                    keep.append(inst)
                elif op == "Matmult":
                    n_mm_since += 1
                    for w in waits:
                        if w.get("sync_type") == "semaphore" and w.get(
                                "wait_mode") == "sem-ge-imm":
                            prev_waits[w["id"]] = max(
                                prev_waits.get(w["id"], 0), w.get("wait_value", 0))
                    keep.append(inst)
                else:
                    # any other PE instruction breaks the pairing window
                    prev_key = None
                    prev_waits = {}
                    n_mm_since = 0
                    keep.append(inst)
            bb["instructions"] = keep
    return ndrop


def _drop_vacuous_waits(j) -> int:
    """Remove sem waits already implied by an earlier wait on the same
    engine queue (same sem, >= value).  Sem values are monotone
    non-decreasing and never reset mid-program, and each engine executes
    its queue in order, so re-waiting on an already-passed threshold is a
    no-op.  EventSemaphore instructions left with no waits are dropped."""
    ndrop = 0
    for f in j.get("functions", []):
        for bb in f.get("blocks", []):
            seen = {}  # (engine, sem id) -> max wait_value already executed
            keep = []
            for inst in bb.get("instructions", []):
                eng = inst.get("engine")
                si = inst.get("sync_info") or {}
                waits = si.get("on_wait") or []
                kept_waits = []
                for w in waits:
                    if (w.get("sync_type") == "semaphore"
                            and w.get("wait_mode") == "sem-ge-imm"):
                        k = (eng, w["id"])
                        v = w.get("wait_value", 0)
                        if v <= seen.get(k, -1):
                            ndrop += 1
                            continue
                        seen[k] = v
                    kept_waits.append(w)
                if si:
                    si["on_wait"] = kept_waits
                if (inst.get("opcode") == "EventSemaphore" and not kept_waits
                        and not (si.get("on_update") or [])):
                    continue
                keep.append(inst)
            bb["instructions"] = keep
    return ndrop


# The remote executor faults at runtime when back-to-back same-weights
# matmuls share one Ldweights (NEFF compiles, execution dies), so the
# dedup/vacuous passes stay disabled.
_LDW_DEDUP = False


def _fix_sync_waits(blob: bytes) -> bytes:
    j = json.loads(blob)
    if _LDW_DEDUP:
        _drop_vacuous_waits(j)
        _dedup_ldweights(j)
    n = 0
    for f in j.get("functions", []):
        for bb in f.get("blocks", []):
            out = []
            for inst in bb.get("instructions", []):
                si = inst.get("sync_info")
                waits = (si or {}).get("on_wait") or []
                cap = _WAIT_CAPS.get(inst.get("opcode"), _DEFAULT_WAIT_CAP)
                if len(waits) > cap:
                    excess, keep = waits[:-cap], waits[-cap:]
                    for w in excess:
                        n += 1
                        out.append({
                            "debug": inst.get("debug", 0),
                            "engine": inst["engine"],
                            "ins": [],
                            "name": f"waitsplit-{n}",
                            "opcode": "EventSemaphore",
                            "outs": [],
                            "sync_info": {"on_update": [], "on_wait": [w]},
                        })
                    si["on_wait"] = keep
                out.append(inst)
            bb["instructions"] = out
    return json.dumps(j).encode()


# ---------------------------------------------------------------------------
_DMA_SRC_CONST = False  # debug ablation: output DMAs read a constant tile
_SKIP_OUT_DMA = False   # debug ablation: no output DMAs (PE/evac floor)
_EVAC_PAT = "VAA"       # evacuation engine rotation: V=DVE, A=ACT
                        # (Pool/gpsimd cannot access PSUM -- walrus birverifier)
                        # ACT copies run ~2x cheaper than DVE here, so ACT
                        # takes 2/3 of the evacuations: paired A/B measured
                        # VAA ~4us/body faster than the 50/50 VA split
_DMA_RINGS = 2          # rotate output stores across SP HWDGE / Pool SWDGE
_STG_BUFS = 3           # output staging buffers (store pipeline depth)
_PAIR_L = False         # score waves: two l's per 4-bank PSUM tile, one
                        # [128,2048] evacuation per pair (halves score evac
                        # instruction count, halves PSUM lookahead to 2)
_SKIP_IN_DMA = False    # debug ablation: no x/fc_w/bi_w loads (timing only)
_TBL_ACT = False        # tT/blT evacs pinned to ACT: wave matmuls then wait
                        # on ONE monotone ACT sem for blT/tT readiness
                        # instead of alternating DVE/ACT sems (fewer split
                        # EventSemaphore instructions on the PE queue)


def _emit_consts(nc, const_pool):
    stg_const = None
    if _DMA_SRC_CONST:
        stg_const = const_pool.tile([128, 6 * 512], BF16, tag="stg_const")
        nc.vector.memset(stg_const[:], 1.0)
    return stg_const


def _emit_body(nc, tc, pools, dram, ctr, consts):
    """Emit one full per-core computation."""
    x_d, fcw_d, fcb_d, biw_d, bias_d, out_d = dram
    (const_pool, big_pool, in_pool, psum_s, psum_w, stg_pool, dram_pool) = pools
    stg_const = consts

    def evac(dst_ap, src_ap, bias_ap=None, force_act=False):
        """PSUM -> SBUF copy (+ optional per-partition bias add), rotated
        across DVE / ACT / Pool per _EVAC_PAT (weighted by engine rates)."""
        c = ctr[0]
        ctr[0] += 1
        eng = "A" if force_act else _EVAC_PAT[c % len(_EVAC_PAT)]
        if eng == "A":
            if bias_ap is not None:
                # Copy doesn't accept an AP bias; Identity does.
                nc.scalar.activation(dst_ap, src_ap, ACT_IDENT, bias=bias_ap)
            else:
                nc.scalar.activation(dst_ap, src_ap, ACT_COPY)
        else:
            if bias_ap is not None:
                nc.vector.tensor_scalar_add(dst_ap, src_ap, bias_ap)
            else:
                nc.vector.tensor_copy(dst_ap, src_ap)

    # ---- persistent SBUF tensors -------------------------------------------
    fcb_sb = const_pool.tile([128, 2], FP32, tag="fcb_sb")      # col ec: fc_b[ec*128+p]
    bias_sb = const_pool.tile([128, 2], FP32, tag="bias_sb")
    xT = big_pool.tile([128, 6 * 1024], BF16, tag="xT")         # [i%128, (i/128, s)]
    fcwT = big_pool.tile([128, 6 * 256], BF16, tag="fcwT")      # [i%128, (i/128, e)]
    biwT = big_pool.tile([128, 2 * 3072], BF16, tag="biwT")     # [e%128, (e/128, f)]
    tT = big_pool.tile([128, 2 * 1024], BF16, tag="tT")         # [e%128, (e/128, s)]
    blT = big_pool.tile([128, 24 * 1024], BF16, tag="blT")      # [f%128, (f/128, s)]

    # ---- input loads --------------------------------------------------------
    # x / fc_w / bi_w arrive HOST-pre-transposed AND pre-cast to bf16 in
    # DRAM (xT [IN,S], fc_wT [IN,E], bi_wT [E,E*L] -- kernel() marshals
    # them), so the transposed SBUF layouts are produced by straight
    # same-dtype DMA loads: no PE transposes, no PSUM round trip, no
    # evacuation copies, half the load bytes of the fp32 upload, and no
    # gpsimd-only cast restriction -- the loads split across the SP HWDGE
    # and Pool SWDGE queues.  Order = startup critical path: x (gates tT),
    # fc_w, bi_w f-half 0 (gates blT f 0-11).
    xT_dst = xT[:].rearrange("p (ic s) -> p ic s", ic=6)
    x_src = x_d.rearrange("(ic p) s -> p ic s", p=128)
    biwT_dst = biwT[:].rearrange("p (kc f) -> p kc f", kc=2)
    biw_src = biw_d.rearrange("(kc p) f -> p kc f", p=128)
    if not _SKIP_IN_DMA:
        nc.sync.dma_start(out=xT_dst[:, :, 0:512], in_=x_src[:, :, 0:512])
        nc.gpsimd.dma_start(
            out=fcwT[:].rearrange("p (ic e) -> p ic e", ic=6),
            in_=fcw_d.rearrange("(ic p) e -> p ic e", p=128))
        nc.sync.dma_start(out=xT_dst[:, :, 512:1024], in_=x_src[:, :, 512:1024])
        nc.gpsimd.dma_start(out=biwT_dst[:, :, 0:1536], in_=biw_src[:, :, 0:1536])
        nc.gpsimd.dma_start(out=biwT_dst[:, :, 1536:3072], in_=biw_src[:, :, 1536:3072])
    nc.sync.dma_start(
        out=fcb_sb[:], in_=fcb_d.rearrange("(c p) one -> p (c one)", p=128))
    nc.sync.dma_start(
        out=bias_sb[:], in_=bias_d.rearrange("(c p) one -> p (c one)", p=128))

    # ---- building blocks ----------------------------------------------------
    # Matmul moving size is N=512: the ISA caps a matmul's PSUM output at one
    # bank (512 fp32).  PSUM tiles are full [128,1024] 2-bank tiles written
    # by matmul pairs (one per bank) so every evacuation is a single
    # full-width [128,1024] instruction.
    def biwT_col(ft, kc):
        return kc * 3072 + ft * 128

    def emit_tT(ec):
        p = psum_s.tile([128, 1024], FP32, tag="pmm")
        for ic in range(6):
            for sh in range(2):
                nc.tensor.matmul(
                    p[:, sh * 512:(sh + 1) * 512],
                    fcwT[:, ic * 256 + ec * 128:ic * 256 + (ec + 1) * 128],
                    xT[:, ic * 1024 + sh * 512:ic * 1024 + (sh + 1) * 512],
                    start=(ic == 0), stop=(ic == 5))
        evac(tT[:, ec * 1024:(ec + 1) * 1024], p[:],
             bias_ap=fcb_sb[:, ec:ec + 1], force_act=_TBL_ACT)

    def emit_blT(fts):
        for ft in fts:
            p = psum_s.tile([128, 1024], FP32, tag="pmm")
            for kc in range(2):
                for sh in range(2):
                    nc.tensor.matmul(
                        p[:, sh * 512:(sh + 1) * 512],
                        biwT[:, biwT_col(ft, kc):biwT_col(ft, kc) + 128],
                        tT[:, kc * 1024 + sh * 512:kc * 1024 + (sh + 1) * 512],
                        start=(kc == 0), stop=(kc == 1))
            evac(blT[:, ft * 1024:(ft + 1) * 1024], p[:],
                 bias_ap=bias_sb[:, ft % 2:ft % 2 + 1], force_act=_TBL_ACT)

    def out_dma(out_ap, in_ap):
        # Rotate output stores across independent descriptor-generation
        # paths (SP HWDGE and the otherwise-idle Pool SWDGE) so trigger /
        # completion handling of consecutive stores proceeds in parallel.
        # ACT is deliberately excluded: a dma trigger's sem-wait executes
        # in-order on the issuing queue and would stall ACT's evac copies.
        if _SKIP_OUT_DMA:
            return
        engines = [nc.sync, nc.gpsimd][:max(1, _DMA_RINGS)]
        eng = engines[ctr[1] % len(engines)]
        ctr[1] += 1
        eng.dma_start(out=out_ap, in_=in_ap)

    def emit_wave(its, lhs=(0, 1), tail_split=False):
        # output unit = (i-tile, l-half) x FULL j: [128 i, 6 l, 1024 j].
        # Full-j units make every partition's DRAM write one contiguous 12KB
        # run -- HW probe showed 2KB-granular strided writes sustain only
        # ~half the bandwidth of contiguous runs.  One l per 2-bank PSUM
        # tile written by four N=512 matmuls (kc accumulation x j-halves),
        # single [128,1024] evacuation.
        # tail_split: ship the last unit as two 3-label DMAs so the final
        # drain overlaps the last evacuations.
        for it in its:
            for lh in lhs:
                last = tail_split and it == its[-1] and lh == lhs[-1]
                stg = stg_pool.tile([128, 6 * 1024], BF16, tag="stg")
                npair = 2 if _PAIR_L else 1
                for lp in range(0, 6, npair):
                    p = psum_w.tile([128, npair * 1024], FP32, tag="pmm")
                    for li in range(npair):
                        l = lh * 6 + lp + li
                        # kc outer: each blT weight tile is loaded once and
                        # streams both j-halves (half the LDWEIGHTS traffic)
                        for kc in range(2):
                            ft = 2 * l + kc
                            for jh in range(2):
                                c0 = li * 1024 + jh * 512
                                nc.tensor.matmul(
                                    p[:, c0:c0 + 512],
                                    blT[:, ft * 1024 + it * 128:ft * 1024 + (it + 1) * 128],
                                    tT[:, kc * 1024 + jh * 512:kc * 1024 + (jh + 1) * 512],
                                    start=(kc == 0), stop=(kc == 1))
                    evac(stg[:, lp * 1024:(lp + npair) * 1024], p[:])
                    cut = 4 if _PAIR_L else 3
                    if last and lp + npair == cut:
                        out_dma(
                            out_d[it * 128:(it + 1) * 128, lh * 6:lh * 6 + cut, :],
                            stg[:, 0:cut * 1024].rearrange("p (l j) -> p l j", l=cut))
                if last:
                    cut = 4 if _PAIR_L else 3
                    out_dma(
                        out_d[it * 128:(it + 1) * 128, lh * 6 + cut:lh * 6 + 6, :],
                        stg[:, cut * 1024:].rearrange("p (l j) -> p l j", l=6 - cut))
                else:
                    out_dma(
                        out_d[it * 128:(it + 1) * 128, lh * 6:lh * 6 + 6, :],
                        stg[:].rearrange("p (l j) -> p l j", l=6))

    # ---- schedule -----------------------------------------------------------
    # tT is produced full-width (both e-halves), then blT f-tiles 0-11
    # (l-half 0 weights) so l-half-0 waves for all 8 i-tiles ship while
    # f-tiles 12-23 are still being produced.
    emit_tT(0)
    emit_tT(1)
    emit_blT(range(0, 12))
    emit_wave((0, 1, 2, 3), lhs=(0,))
    emit_blT(range(12, 24))
    emit_wave((4, 5, 6, 7), lhs=(0,))
    emit_wave((0, 1, 2, 3), lhs=(1,))
    emit_wave((4, 5, 6, 7), lhs=(1,), tail_split=True)


def build_nc(unroll: int = 1):
    """Build the Bass program.  unroll>1 repeats the whole body (for timing
    measurements via wall-clock differencing)."""
    nc = bass.Bass(trn_type="TRN2")
    # x / fc_w / bi_w are uploaded host-pre-transposed and pre-cast to bf16
    # (see _emit_body loads)
    x_d = nc.dram_tensor("x", [IN, S], BF16, kind="ExternalInput")
    fcw_d = nc.dram_tensor("fc_w", [IN, E], BF16, kind="ExternalInput")
    fcb_d = nc.dram_tensor("fc_b", [E, 1], FP32, kind="ExternalInput")
    biw_d = nc.dram_tensor("bi_w", [E, E * L], BF16, kind="ExternalInput")
    bias_d = nc.dram_tensor("bias", [E, 1], FP32, kind="ExternalInput")
    # Output is stored bf16 (halves the dominant HBM write stream); the host
    # upcasts to fp32.  Quantization adds ~1e-3 rel err on top of the ~4e-3
    # bf16-compute error -- well inside the 2e-2 gate.
    out_d = nc.dram_tensor("out", [S, L, S], BF16, kind="ExternalOutput")
    dram = (x_d, fcw_d, fcb_d, biw_d, bias_d, out_d)

    with tile.TileContext(nc) as tc:
        with (
            tc.tile_pool(name="const", bufs=1) as const_pool,
            tc.tile_pool(name="big", bufs=1) as big_pool,
            tc.tile_pool(name="inp", bufs=1) as in_pool,
            tc.tile_pool(name="psum_mm", bufs=2 if _PAIR_L else 4,
                         space="PSUM") as psum_mm,
            tc.tile_pool(name="stg", bufs=_STG_BUFS) as stg_pool,
            tc.tile_pool(name="dram", bufs=1, space="DRAM") as dram_pool,
        ):
            pools = (const_pool, big_pool, in_pool, psum_mm, psum_mm, stg_pool,
                     dram_pool)
            ctr = [0, 0]
            consts = _emit_consts(nc, const_pool)
            for _ in range(unroll):
                _emit_body(nc, tc, pools, dram, ctr, consts)

    blob = _fix_sync_waits(nc.to_json_bytes())
    nc.to_json_bytes = lambda: blob
    return nc


_CACHE = {}


def _get_nc(unroll: int = 1):
    if unroll not in _CACHE:
        _CACHE[unroll] = build_nc(unroll)
    return _CACHE[unroll]


def kernel(input_tensor, fc_w, fc_b, bi_w, bias):
    # Host-side input marshaling: per-core batch slice of x, and x/fc_w/bi_w
    # pre-transposed AND pre-cast to bf16 so the device consumes them with
    # straight same-dtype strided loads (host cast is bitwise-identical to
    # the previous on-device DMA cast).
    import ml_dtypes
    bf16 = ml_dtypes.bfloat16
    input_tensor = np.asarray(input_tensor, dtype=np.float32)
    fcw_t = np.ascontiguousarray(np.asarray(fc_w, dtype=np.float32).T.astype(bf16))
    fc_b = np.ascontiguousarray(np.asarray(fc_b, dtype=np.float32)).reshape(E, 1)
    biw_t = np.ascontiguousarray(np.asarray(bi_w, dtype=np.float32).T.astype(bf16))
    bias = np.ascontiguousarray(np.asarray(bias, dtype=np.float32)).reshape(E, 1)
    assert input_tensor.shape == (B, S, IN)

    nc = _get_nc()
    in_maps = [
        {"x": np.ascontiguousarray(input_tensor[c].T.astype(bf16)), "fc_w": fcw_t,
         "fc_b": fc_b, "bi_w": biw_t, "bias": bias}
        for c in range(N_CORES)
    ]
    res = run_bass_kernel_spmd(nc, in_maps, core_ids=list(range(N_CORES)))
    return np.stack(
        [np.asarray(res.results[c]["out"]) for c in range(N_CORES)], axis=0
    ).astype(np.float32)

